# revision 9
# baseline (speedup 1.0000x reference)
"""CrossAttentionBridge Trainium2 kernel.

Sharding: data-parallel over batch B=8 -> one batch element per NeuronCore.
Each core runs the full bridge (LN -> QKV -> RoPE -> attention+bias ->
out-proj -> sigmoid gate -> residual mix) on its [512, 1024] slice; no
collectives.

Layout: activations are kept transposed ([d_model, tokens]) so every matmul
contraction lands on the partition dim. Scores are computed as
scores^T [k_tok, q_tok] so softmax needs no transposes: exp runs on ACT
straight out of PSUM, the denominator comes from an extra ones-column
appended to V, and the divide is deferred to after the attn@V matmul.
Head dims of Q/K are interleaved host-side (even/odd pairs adjacent) so the
RoPE half-rotation becomes a partition-stride-2 access instead of a 32-row
block swap.
"""

import numpy as np

D = 1024
HEADS = 16
HD = 64
L = 512
P = 128
DT = D // P           # 8 d_model tiles
TT = L // P           # 4 token tiles
SCALE = HD ** -0.5
BIAS_LEN = 128
ROPE_BASE = 10000.0
LN_EPS = 1e-5
N_CORES = 8

_CACHE = {}


# ---------------------------------------------------------------- host prep

def _np_temporal_bias():
    pos = np.arange(BIAS_LEN, dtype=np.float32)
    dist = np.abs(pos[None, :] - pos[:, None])
    bias = np.exp(-dist * 0.1) - dist * 0.05

    def resize1d(a, out_len, axis):
        in_len = a.shape[axis]
        scale = in_len / out_len
        x = (np.arange(out_len, dtype=np.float64) + 0.5) * scale - 0.5
        x0 = np.floor(x).astype(np.int64)
        w = (x - x0).astype(np.float32)
        a0 = np.take(a, np.clip(x0, 0, in_len - 1), axis=axis)
        a1 = np.take(a, np.clip(x0 + 1, 0, in_len - 1), axis=axis)
        shape = [1] * a.ndim
        shape[axis] = out_len
        return a0 * (1 - w.reshape(shape)) + a1 * w.reshape(shape)

    out = resize1d(bias, L, 0)
    out = resize1d(out, L, 1)
    return out.astype(np.float32)


def _rope_tables():
    # Head-dim rows are permuted host-side: new row 2i <- old i (even half),
    # new row 2i+1 <- old 32+i (odd half). Pair (2i, 2i+1) rotates together.
    inv_freq = 1.0 / ROPE_BASE ** (np.arange(0, HD, 2, dtype=np.float32) / HD)
    t = np.arange(L, dtype=np.float32)
    freqs = t[None, :] * inv_freq[:, None]          # [32, 512]
    cos = np.cos(freqs).astype(np.float32)
    sin = np.sin(freqs).astype(np.float32)
    cos_rows = np.empty((HD, L), np.float32)
    ssin_rows = np.empty((HD, L), np.float32)
    cos_rows[0::2] = cos
    cos_rows[1::2] = cos
    ssin_rows[0::2] = -sin
    ssin_rows[1::2] = sin
    cos_t = np.tile(cos_rows, (2, 1))               # [128, 512]
    ssin_t = np.tile(ssin_rows, (2, 1))
    return cos_t, ssin_t


def _head_perm():
    # permutation of the 64 head dims: [0, 32, 1, 33, ...]
    perm = np.empty(HD, np.int64)
    perm[0::2] = np.arange(32)
    perm[1::2] = np.arange(32) + 32
    return perm


def _prep_shared(Wqkv, Wout, b_out, Wgate, b_gate, gamma, beta):
    perm = _head_perm()
    Wg = (Wqkv * gamma[None, :]).astype(np.float32)
    bqkv = (Wqkv @ beta).astype(np.float32)
    # permute q/k output dims per head (v untouched)
    full_perm = np.arange(3 * D)
    for h in range(HEADS):
        full_perm[h * HD:(h + 1) * HD] = h * HD + perm
        full_perm[D + h * HD:D + (h + 1) * HD] = D + h * HD + perm
    Wg = Wg[full_perm]
    bqkv = bqkv[full_perm]

    cos_t, ssin_t = _rope_tables()
    shared = {
        "wqkv_t": np.ascontiguousarray(Wg.T),
        "bqkv": bqkv,
        "wout_t": np.ascontiguousarray((Wout * 1.0).T).astype(np.float32),
        "bout": b_out.astype(np.float32),
        "wgate_t": np.ascontiguousarray(Wgate.T).astype(np.float32),
        "bgate": b_gate.astype(np.float32),
        "cos_q": (cos_t * SCALE).astype(np.float32),
        "ssin_q": (ssin_t * SCALE).astype(np.float32),
        "cos_k": cos_t,
        "ssin_k": ssin_t,
        "bias_t": np.ascontiguousarray(_np_temporal_bias().T),
    }
    qk = bqkv[:2 * D].reshape(-1, 2)[:, ::-1].reshape(-1)  # swap pairs (2i,2i+1)
    shared["bqkv_sw"] = np.ascontiguousarray(qk)
    return shared


def _to_bf16(x):
    import ml_dtypes
    return x.astype(ml_dtypes.bfloat16)


# ---------------------------------------------------------------- bass build

def _build_nc():
    from contextlib import ExitStack
    import concourse.bass as bass
    import concourse.tile as tile
    from concourse import mybir, bacc, library_config

    f32 = mybir.dt.float32
    f32r = mybir.dt.float32r
    bf16 = mybir.dt.bfloat16
    Alu = mybir.AluOpType
    Act = mybir.ActivationFunctionType

    nc = bacc.Bacc("TRN2", target_bir_lowering=False, debug=False,
                   num_devices=N_CORES)

    xdec_t = nc.declare_dram_parameter("xdec_t", [D, L], f32, isOutput=False)
    xenc_t = nc.declare_dram_parameter("xenc_t", [D, L], f32, isOutput=False)
    wqkv_t = nc.declare_dram_parameter("wqkv_t", [D, 3 * D], bf16, isOutput=False)
    wout_t = nc.declare_dram_parameter("wout_t", [D, D], bf16, isOutput=False)
    wgate_t = nc.declare_dram_parameter("wgate_t", [D, D], bf16, isOutput=False)
    bqkv = nc.declare_dram_parameter("bqkv", [3 * D], f32, isOutput=False)
    bqkv_sw = nc.declare_dram_parameter("bqkv_sw", [2 * D], f32, isOutput=False)
    bout = nc.declare_dram_parameter("bout", [D], f32, isOutput=False)
    bgate = nc.declare_dram_parameter("bgate", [D], f32, isOutput=False)
    cos_q = nc.declare_dram_parameter("cos_q", [P, L], f32, isOutput=False)
    ssin_q = nc.declare_dram_parameter("ssin_q", [P, L], f32, isOutput=False)
    cos_k = nc.declare_dram_parameter("cos_k", [P, L], f32, isOutput=False)
    ssin_k = nc.declare_dram_parameter("ssin_k", [P, L], f32, isOutput=False)
    bias_t = nc.declare_dram_parameter("bias_t", [L, L], f32, isOutput=False)
    out_t = nc.declare_dram_parameter("out_t", [D, L], f32, isOutput=True)

    VW = HD + 1  # v columns per head incl. ones column

    with tile.TileContext(nc) as tc, ExitStack() as ctx:
        nc.gpsimd.load_library(library_config.attn)

        const = ctx.enter_context(tc.tile_pool(name="const", bufs=1))
        small = ctx.enter_context(tc.tile_pool(name="small", bufs=6))
        bcln_pool = ctx.enter_context(tc.tile_pool(name="bcln", bufs=4))
        bch_pool = ctx.enter_context(tc.tile_pool(name="bch", bufs=2))
        xdec_pool = ctx.enter_context(tc.tile_pool(name="xdec", bufs=DT))
        # xenc slots are recycled by the att2 tiles in phase D (same tag)
        xenc_pool = ctx.enter_context(tc.tile_pool(name="xenc", bufs=DT))
        xhat_pool = ctx.enter_context(tc.tile_pool(name="xhat", bufs=DT))
        scratch = ctx.enter_context(tc.tile_pool(name="scratch", bufs=5))
        wq_pool = ctx.enter_context(tc.tile_pool(name="wq", bufs=8))
        wo_pool = ctx.enter_context(tc.tile_pool(name="wo", bufs=DT))
        wg_pool = ctx.enter_context(tc.tile_pool(name="wg", bufs=DT))
        rope_pool = ctx.enter_context(tc.tile_pool(name="rope", bufs=DT))
        v1_pool = ctx.enter_context(tc.tile_pool(name="v1", bufs=TT))
        att_pool = ctx.enter_context(tc.tile_pool(name="attT", bufs=DT))
        fin_pool = ctx.enter_context(tc.tile_pool(name="fin", bufs=5))

        ps_stats = ctx.enter_context(tc.tile_pool(name="ps_stats", bufs=2, space="PSUM"))
        ps_mm = ctx.enter_context(tc.tile_pool(name="ps_mm", bufs=4, space="PSUM"))
        ps_att = ctx.enter_context(tc.tile_pool(name="ps_att", bufs=2, space="PSUM"))

        # ---- constants
        ones_col = const.tile([P, 1], bf16)
        nc.vector.memset(ones_col[:], 1.0)
        eps_col = const.tile([P, 1], f32)
        nc.vector.memset(eps_col[:], LN_EPS)

        cosq_sb = const.tile([P, L], f32)
        nc.sync.dma_start(out=cosq_sb[:], in_=cos_q[:, :])
        ssinq_sb = const.tile([P, L], f32)
        nc.sync.dma_start(out=ssinq_sb[:], in_=ssin_q[:, :])
        cosk_sb = const.tile([P, L], f32)
        nc.sync.dma_start(out=cosk_sb[:], in_=cos_k[:, :])
        ssink_sb = const.tile([P, L], f32)
        nc.sync.dma_start(out=ssink_sb[:], in_=ssin_k[:, :])

        biasT_sb = []
        for i in range(TT):
            t = const.tile([P, L], f32, tag=f"biasT{i}")
            nc.sync.dma_start(out=t[:], in_=bias_t[i * P:(i + 1) * P, :])
            biasT_sb.append(t)

        bq_sb = const.tile([P, 5 * DT], f32)
        nc.sync.dma_start(out=bq_sb[:, 0:3 * DT],
                          in_=bqkv.rearrange("(j p) -> p j", p=P))
        nc.sync.dma_start(out=bq_sb[:, 3 * DT:5 * DT],
                          in_=bqkv_sw.rearrange("(j p) -> p j", p=P))
        bout_sb = const.tile([P, DT], f32)
        nc.sync.dma_start(out=bout_sb[:], in_=bout.rearrange("(j p) -> p j", p=P))
        bgate_sb = const.tile([P, DT], f32)
        nc.sync.dma_start(out=bgate_sb[:], in_=bgate.rearrange("(j p) -> p j", p=P))
        vbias_bc = const.tile([P, D], bf16)
        nc.gpsimd.dma_start(out=vbias_bc[:], in_=bqkv[None, 2 * D:3 * D].to_broadcast([P, D]))

        # ---- inputs
        xdec = []
        xenc = []
        for k in range(DT):
            td = xdec_pool.tile([P, L], f32, tag="xdec")
            nc.sync.dma_start(out=td[:], in_=xdec_t[k * P:(k + 1) * P, :])
            xdec.append(td)
            te = xenc_pool.tile([P, L], f32, tag="xenc")
            nc.sync.dma_start(out=te[:], in_=xenc_t[k * P:(k + 1) * P, :])
            xenc.append(te)

        # prefetch out/gate weights (used in phase D)
        wout_sb = []
        wgate_sb = []
        for k in range(DT):
            t = wo_pool.tile([P, D], bf16, tag="wo")
            nc.sync.dma_start(out=t[:], in_=wout_t[k * P:(k + 1) * P, :])
            wout_sb.append(t)
            t = wg_pool.tile([P, D], bf16, tag="wg")
            nc.sync.dma_start(out=t[:], in_=wgate_t[k * P:(k + 1) * P, :])
            wgate_sb.append(t)

        # ---- phase A: layernorm -> xhat (bf16, transposed layout)
        def layernorm(xtiles, tag):
            ps_sum = ps_stats.tile([1, L], f32, tag="stats")
            xb_tiles = []
            for k in range(DT):
                xb = scratch.tile([P, L], bf16, tag="scr")
                nc.vector.tensor_copy(xb[:], xtiles[k][:])
                xb_tiles.append(xb)
                nc.tensor.matmul(ps_sum[:], ones_col[:], xb[:],
                                 start=(k == 0), stop=(k == DT - 1))
            ps_sq = ps_stats.tile([1, L], f32, tag="stats")
            for k in range(DT):
                sq = scratch.tile([P, L], bf16, tag="scr")
                nc.scalar.square(sq[:], xtiles[k][:])
                nc.tensor.matmul(ps_sq[:], ones_col[:], sq[:],
                                 start=(k == 0), stop=(k == DT - 1))
            mu = small.tile([1, L], f32, tag="sm")
            nc.scalar.mul(mu[:], ps_sum[:], 1.0 / D)
            m2 = small.tile([1, L], f32, tag="sm")
            nc.scalar.mul(m2[:], ps_sq[:], 1.0 / D)
            var = small.tile([1, L], f32, tag="sm")
            # var = m2 - mu*mu  ->  (mu * -mu)?? use two ops
            mu2 = small.tile([1, L], f32, tag="sm")
            nc.vector.tensor_tensor(mu2[:], mu[:], mu[:], Alu.mult)
            nc.vector.tensor_tensor(var[:], m2[:], mu2[:], Alu.subtract)
            std = small.tile([1, L], f32, tag="sm")
            nc.scalar.activation(std[:], var[:], Act.Sqrt, bias=eps_col[0:1, :])
            rstd = small.tile([1, L], f32, tag="sm")
            nc.vector.reciprocal(rstd[:], std[:])
            nmr = small.tile([1, L], f32, tag="sm")
            nc.vector.scalar_tensor_tensor(nmr[:], mu[:], -1.0, rstd[:],
                                           op0=Alu.mult, op1=Alu.mult)
            rstd_bc = bcln_pool.tile([P, L], f32, tag="bc")
            nc.gpsimd.partition_broadcast(rstd_bc[:], rstd[:])
            nmr_bc = bcln_pool.tile([P, L], f32, tag="bc")
            nc.gpsimd.partition_broadcast(nmr_bc[:], nmr[:])
            out = []
            for k in range(DT):
                t = scratch.tile([P, L], f32, tag="scr")
                nc.vector.tensor_tensor(t[:], xtiles[k][:], rstd_bc[:], Alu.mult)
                xh = xhat_pool.tile([P, L], bf16, tag=f"xhat_{tag}")
                nc.vector.tensor_tensor(xh[:], t[:], nmr_bc[:], Alu.add)
                out.append(xh)
            return out

        xhat_dec = layernorm(xdec, "d")
        xhat_enc = layernorm(xenc, "e")

        # ---- phase B: QKV + RoPE
        SWAP_MASK = [i ^ 1 for i in range(32)]

        def qkv_rope(xhat, col0, cos_sb, ssin_sb, bias_col0, bias_swap0, rtag):
            """project + rope; returns 8 bf16 [128,512] tiles (rows=dims)."""
            out_tiles = []
            # stage the needed weight slices
            w_sl = []
            for k in range(DT):
                w = wq_pool.tile([P, D], bf16, tag="wqslot")
                nc.sync.dma_start(out=w[:], in_=wqkv_t[k * P:(k + 1) * P,
                                                       col0:col0 + D])
                w_sl.append(w)
            for j in range(DT):
                ps = ps_mm.tile([P, L], f32, tag="mm")
                for k in range(DT):
                    nc.tensor.matmul(ps[:], w_sl[k][:, j * P:(j + 1) * P],
                                     xhat[k][:],
                                     start=(k == 0), stop=(k == DT - 1))
                bcol = bq_sb[:, (bias_col0 + j):(bias_col0 + j) + 1]
                bswap = bq_sb[:, (bias_swap0 + j):(bias_swap0 + j) + 1]
                # pair partner via per-quadrant partition shuffle (d XOR 1)
                qs = scratch.tile([P, L], f32, tag="scr")
                nc.vector.stream_shuffle(qs[:], ps[:], mask=SWAP_MASK)
                m1 = scratch.tile([P, L], f32, tag="scr")
                nc.vector.scalar_tensor_tensor(m1[:], ps[:], bcol, cos_sb[:],
                                               op0=Alu.add, op1=Alu.mult)
                m2 = scratch.tile([P, L], f32, tag="scr")
                nc.vector.scalar_tensor_tensor(m2[:], qs[:], bswap, ssin_sb[:],
                                               op0=Alu.add, op1=Alu.mult)
                ro = rope_pool.tile([P, L], bf16, tag=rtag)
                nc.vector.tensor_tensor(ro[:], m1[:], m2[:], Alu.add)
                out_tiles.append(ro)
            return out_tiles

        q_rope = qkv_rope(xhat_dec, 0, cosq_sb, ssinq_sb, 0, 3 * DT, "rq")
        k_rope = qkv_rope(xhat_enc, D, cosk_sb, ssink_sb, DT, 4 * DT, "rk")

        # ---- v (natural layout [tok, head*65]) with ones columns
        v1 = []
        for i in range(TT):
            v = v1_pool.tile([P, HEADS * VW], bf16, tag="v1", name=f"v1_{i}")
            v1.append(v)
        vw_sl = []
        for k in range(DT):
            w = wq_pool.tile([P, D], bf16, tag="wqslot")
            nc.sync.dma_start(out=w[:], in_=wqkv_t[k * P:(k + 1) * P,
                                                   2 * D:3 * D])
            vw_sl.append(w)
        for i in range(TT):
            # ones columns (stride VW starting at col 64)
            ones_view = v1[i].rearrange("p (h w) -> p h w", w=VW)[:, :, HD:]
            nc.vector.memset(ones_view, 1.0)
            for nh in range(2):
                ps = ps_mm.tile([P, L], f32, tag="mm")
                for k in range(DT):
                    nc.tensor.matmul(ps[:], xhat_enc[k][:, i * P:(i + 1) * P],
                                     vw_sl[k][:, nh * L:(nh + 1) * L],
                                     start=(k == 0), stop=(k == DT - 1))
                out_view = v1[i].rearrange("p (h w) -> p h w", w=VW)[
                    :, nh * 8:(nh + 1) * 8, 0:HD]
                ps_view = ps[:].rearrange("p (h d) -> p h d", d=HD)
                vb_view = vbias_bc[:, nh * L:(nh + 1) * L].rearrange(
                    "p (h d) -> p h d", d=HD)
                nc.vector.tensor_tensor(out_view, ps_view, vb_view, Alu.add)

        # ---- phase C: attention per head
        attT = []
        for j in range(DT):
            attT.append(att_pool.tile([P, L], bf16, tag="attT", name=f"attT{j}"))
        for h in range(HEADS):
            j, r = h // 2, (h % 2) * HD
            exp_tiles = []
            for i in range(TT):
                ps_s = ps_mm.tile([P, L], f32, tag="mm")
                nc.tensor.matmul(ps_s[:],
                                 k_rope[j][r:r + HD, i * P:(i + 1) * P],
                                 q_rope[j][r:r + HD, :],
                                 start=True, stop=True)
                nc.vector.tensor_tensor(ps_s[:], ps_s[:], biasT_sb[i][:], Alu.add)
                e = wq_pool.tile([P, L], bf16, tag="wqslot")
                nc.scalar.activation(e[:], ps_s[:], Act.Exp)
                exp_tiles.append(e)
            ps_a = ps_att.tile([VW, L], f32, tag="att")
            for i in range(TT):
                nc.tensor.matmul(ps_a[:], v1[i][:, h * VW:(h + 1) * VW],
                                 exp_tiles[i][:],
                                 start=(i == 0), stop=(i == TT - 1))
            recip = small.tile([1, L], f32, tag="sm")
            nc.vector.reciprocal(recip[:], ps_a[HD:HD + 1, :])
            rb = bch_pool.tile([HD, L], f32, tag="bch")
            nc.gpsimd.partition_broadcast(rb[:], recip[:], channels=HD)
            nc.vector.tensor_tensor(attT[j][r:r + HD, :], ps_a[0:HD, :], rb[:],
                                    Alu.mult)

        # ---- phase D: out proj, gate, combine
        att2 = []
        for j in range(DT):
            ps = ps_mm.tile([P, L], f32, tag="mm")
            for k in range(DT):
                nc.tensor.matmul(ps[:], wout_sb[k][:, j * P:(j + 1) * P],
                                 attT[k][:],
                                 start=(k == 0), stop=(k == DT - 1))
            a2 = xenc_pool.tile([P, L], bf16, tag="xenc")
            nc.vector.tensor_scalar(a2[:], ps[:], bout_sb[:, j:j + 1], None,
                                    op0=Alu.add)
            att2.append(a2)
        for j in range(DT):
            ps = ps_mm.tile([P, L], f32, tag="mm")
            for k in range(DT):
                nc.tensor.matmul(ps[:], wgate_sb[k][:, j * P:(j + 1) * P],
                                 att2[k][:],
                                 start=(k == 0), stop=(k == DT - 1))
            gate = fin_pool.tile([P, L], f32, tag="fin")
            nc.scalar.activation(gate[:], ps[:], Act.Sigmoid,
                                 bias=bgate_sb[:, j:j + 1])
            # out = res + gate * (att2 - res)
            dlt = fin_pool.tile([P, L], f32, tag="fin")
            nc.vector.tensor_tensor(dlt[:], att2[j][:], xdec[j][:], Alu.subtract)
            gd = fin_pool.tile([P, L], f32, tag="fin")
            nc.vector.tensor_tensor(gd[:], gate[:], dlt[:], Alu.mult)
            ot = fin_pool.tile([P, L], f32, tag="fin")
            nc.vector.tensor_tensor(ot[:], gd[:], xdec[j][:], Alu.add)
            nc.sync.dma_start(out=out_t[j * P:(j + 1) * P, :], in_=ot[:])

    nc.compile()
    return nc


def get_nc():
    if "nc" not in _CACHE:
        _CACHE["nc"] = _build_nc()
    return _CACHE["nc"]


def make_in_maps(inputs):
    shared = _prep_shared(inputs["Wqkv"], inputs["Wout"], inputs["b_out"],
                          inputs["Wgate"], inputs["b_gate"],
                          inputs["gamma"], inputs["beta"])
    shared["wqkv_t"] = _to_bf16(shared["wqkv_t"])
    shared["wout_t"] = _to_bf16(shared["wout_t"])
    shared["wgate_t"] = _to_bf16(shared["wgate_t"])
    dec = np.asarray(inputs["decoder_hidden"], np.float32)
    enc = np.asarray(inputs["encoder_output"], np.float32)
    in_maps = []
    for b in range(N_CORES):
        m = dict(shared)
        m["xdec_t"] = np.ascontiguousarray(dec[b].T)
        m["xenc_t"] = np.ascontiguousarray(enc[b].T)
        in_maps.append(m)
    return in_maps


def kernel(**inputs):
    from concourse.bass_utils import run_bass_kernel_spmd
    nc = get_nc()
    in_maps = make_in_maps(inputs)
    res = run_bass_kernel_spmd(nc, in_maps, core_ids=list(range(N_CORES)))
    out = np.stack([res.results[b]["out_t"].T for b in range(N_CORES)])
    return np.ascontiguousarray(out.astype(np.float32))
